# revision 22
# baseline (speedup 1.0000x reference)
"""Trainium2 Bass kernel for nn_DecoderLayer_84404697301735.

3-sublayer decoder (self-attn w/ char rel-pos, cross-attn to char encoder
w/ rel-pos, cross-attn to word encoder w/ word-level pos) + FFN.

Sharding: 8 cores = 4 batch x 2 interleaved query-tile halves.  Each core
computes 512 query rows end-to-end (feature-major layout); K/V projections
over the full 1024 keys are duplicated within a batch pair.  No collectives.

Relative-position logits: Band[i,t] = Q[i] . E[t] via matmul against a
host-built extended pos table E (clip folded in, causal mask folded into a
65th ones-row of Q), then a skewed-stride DMA read from a DRAM round-trip
converts band (query-relative) layout to absolute key layout.  The per-core
query-offset parity is folded into E so the program is core-independent.
"""

import numpy as np
import ml_dtypes

import concourse.bass as bass
import concourse.tile as tile
from concourse import bacc, mybir
from concourse.bass_utils import run_bass_kernel_spmd

BF16 = ml_dtypes.bfloat16
F32 = mybir.dt.float32
F32R = mybir.dt.float32r
BF = mybir.dt.bfloat16

D = 1024
H = 16
DH = 64
S_OWN = 512          # own query rows per core
LK = 1024            # keys
DFF = 4096
M = 128              # pos clip radius
WE = 640             # extended pos table width (r = c - 255 - 128*pi)
SCALE = float(DH) ** 0.5   # 8.0
EPS = 1e-5
NEG = -1e30

AL = mybir.AluOpType
AF = mybir.ActivationFunctionType

# bias_cat column layout (each unit = 1 col of [128, n] per-partition stripes)
_BIAS_SECTS = [
    ("qkv", 24), ("o1", 8), ("q2", 8), ("k2", 8), ("v2", 8), ("o2", 8),
    ("q3", 8), ("k3", 8), ("v3", 8), ("o3", 8), ("f1", 32), ("f2", 8),
    ("ln1g", 8), ("ln1b", 8), ("ln2g", 8), ("ln2b", 8), ("ln3g", 8), ("ln3b", 8),
]
BIAS_COL = {}
_c = 0
for _n, _w in _BIAS_SECTS:
    BIAS_COL[_n] = _c
    _c += _w
NBIAS = _c  # 184


def _emit(nc, tc, ctx, T, debug=False):
    """Emit the whole per-core program.  T: dict name -> dram AP."""
    te, ve, sc, gp, sy = nc.tensor, nc.vector, nc.scalar, nc.gpsimd, nc.sync

    singles = ctx.enter_context(tc.tile_pool(name="singles", bufs=1))
    psum = ctx.enter_context(tc.tile_pool(name="psum", bufs=1, space="PSUM"))
    wpool = ctx.enter_context(tc.tile_pool(name="wpool", bufs=2))
    work = ctx.enter_context(tc.tile_pool(name="work", bufs=3))
    ppool = ctx.enter_context(tc.tile_pool(name="ppool", bufs=2))
    dram = ctx.enter_context(tc.tile_pool(name="dramp", bufs=4, space="DRAM"))
    smalls = ctx.enter_context(tc.tile_pool(name="smalls", bufs=2))
    # PSUM budget (8 banks): mm=3, s=2, pv=2, tp=1

    # ---- persistent SBUF ----
    bias_sb = singles.tile([128, NBIAS], F32)
    sy.dma_start(bias_sb, T["bias"])
    e1t_sb = singles.tile([128, WE], BF)
    sy.dma_start(e1t_sb, T["e1t"])
    e2t_sb = singles.tile([128, WE], BF)
    sy.dma_start(e2t_sb, T["e2t"])
    e1m_sb = singles.tile([1, WE], BF)
    sy.dma_start(e1m_sb, T["e1m"])
    g3t_sb = singles.tile([128, LK], BF)
    sy.dma_start(g3t_sb, T["g3t"])
    eps_sb = singles.tile([1, 1], F32)
    ve.memset(eps_sb, EPS)
    ones_sb = singles.tile([128, 1], BF)
    ve.memset(ones_sb, 1.0)

    xres = singles.tile([128, 8, S_OWN], F32)      # residual stream (feature-major)
    sy.dma_start(xres, T["xow"].rearrange("(a p) r -> p a r", p=128))
    xbf = singles.tile([128, 8, S_OWN], BF)        # bf16 copy for proj rhs
    sy.dma_start(xbf, T["xob"].rearrange("(a p) r -> p a r", p=128))
    big = singles.tile([128, 32, S_OWN], BF)       # enc (sublayers) / FFN hidden
    enc = big.rearrange("p a r -> p (a r)")[:, 0:8 * LK].rearrange(
        "p (a r) -> p a r", a=8)                   # current sublayer's enc input
    kt_all = singles.tile([128, 8, LK], BF)        # K^T, head h at rows 64*(h%2)

    def kth(h):
        return kt_all[64 * (h % 2):64 * (h % 2) + 64, h // 2, :]
    v_all = singles.tile([128, H, 8, 65], BF)      # V key-major + ones col
    gp.memset(v_all, 1.0)
    qt_pk = singles.tile([128, 8, S_OWN], BF)      # Q^T, head h at rows 64*(h%2)

    def qth(h):
        return qt_pk[64 * (h % 2):64 * (h % 2) + 64, h // 2, :]

    onesrow = singles.tile([1, 128], BF)
    gp.memset(onesrow, 1.0)
    aT = singles.tile([128, 8, S_OWN], BF)         # attention output (feature-major)
    h1 = big                                       # FFN hidden (aliases enc)
    ident64 = singles.tile([64, 64], BF)
    ident128 = singles.tile([128, 128], BF)
    from concourse.masks import make_identity
    make_identity(nc, ident64)
    make_identity(nc, ident128)

    def load_enc(name):
        sy.dma_start(enc, T[name].rearrange("(a p) r -> p a r", p=128))

    def bias_ap(col, base=0, size=128):
        return bias_sb[base:base + size, col:col + 1]

    # ---------------- projections ----------------
    # weights in DRAM are ct-major: [nct, nkt, 128, 128]; load a group of
    # ct columns in one DMA strip to amortize per-DMA issue cost.
    def proj(wname, ct0, nct, bias0, rhs_fn, nrc, nkt, epilogue):
        wd = T[wname]
        cchunk = max(1, 16 // nkt)  # ~2048 cols per strip
        for cg in range(0, nct, cchunk):
            ncg = min(cchunk, nct - cg)
            ws = wpool.tile([128, ncg * nkt * 128], BF, tag="w",
                            name=f"ws_{wname}_{ct0}_{cg}")
            src = wd[ct0 + cg:ct0 + cg + ncg].rearrange("a k p c -> p (a k) c")
            sy.dma_start(ws.rearrange("p (a c) -> p a c", c=128), src)
            for ci in range(ncg):
                ct = cg + ci
                pss = []
                for rc in range(nrc):
                    ps = psum.tile([128, 512], F32, tag="mm", bufs=3,
                                   name=f"ps_{wname}_{ct0}_{ct}_{rc}")
                    pss.append(ps)
                for kt in range(nkt):
                    wt = ws[:, (ci * nkt + kt) * 128:(ci * nkt + kt + 1) * 128]
                    for rc in range(nrc):
                        te.matmul(pss[rc], lhsT=wt, rhs=rhs_fn(kt, rc),
                                  start=(kt == 0), stop=(kt == nkt - 1))
                for rc in range(nrc):
                    epilogue(ct, rc, pss[rc])

    def rhs_own(kt, rc):
        return xbf[:, kt, :]

    def rhs_enc(kt, rc):
        return enc[:, kt, 512 * rc:512 * rc + 512]

    def q_ep(bias0):
        def ep(ct, rc, ps):
            for sub in range(2):
                h = 2 * ct + sub
                sc.activation(qth(h), ps[64 * sub:64 * sub + 64, :], AF.Identity,
                              bias=bias_ap(bias0 + ct, 64 * sub, 64))
        return ep

    def k_ep(bias0, add_g):
        def ep(ct, rc, ps):
            for sub in range(2):
                h = 2 * ct + sub
                dst = kth(h)[:, 512 * rc:512 * rc + 512]
                src = ps[64 * sub:64 * sub + 64, :]
                b = bias_ap(bias0 + ct, 64 * sub, 64)
                if add_g:
                    ve.scalar_tensor_tensor(
                        dst, src, b, g3t_sb[64 * sub:64 * sub + 64, 512 * rc:512 * rc + 512],
                        op0=AL.add, op1=AL.add)
                else:
                    sc.activation(dst, src, AF.Identity, bias=b)
        return ep

    def v_ep(bias0):
        def ep(ct, rc, ps):
            for sub in range(2):
                h = 2 * ct + sub
                vt = work.tile([64, 512], BF, tag="vt", name=f"vt_{h}_{rc}")
                sc.activation(vt, ps[64 * sub:64 * sub + 64, :], AF.Identity,
                              bias=bias_ap(bias0 + ct, 64 * sub, 64))
                for j in range(4):
                    tp = psum.tile([128, 64], BF, tag="s", bufs=2,
                                   name=f"vtp_{h}_{rc}_{j}")
                    te.transpose(tp, vt[:, 128 * j:128 * j + 128], ident64)
                    ve.tensor_copy(v_all[:, h, 4 * rc + j, 0:64], tp)
        return ep

    # ---------------- attention (transposed scores) ----------------
    # pT[kb] = exp((K_kb Q^T)/s) [keys, queries]; PV consumes pT directly so
    # P is never transposed.  Relative-position factors via the identity
    # exp(S+B) = exp(S)*exp(B), divided through per query by the left-clip
    # factor: fully-left key tiles need no fixup; a 4-key-tile window around
    # the diagonal is recomputed in [q,k] orientation with the band added via
    # one batched skew-DMA per head, exp'd, and PE-transposed into pT;
    # fully-right tiles are memset to 0 (causal sl1) or scaled by the
    # per-query right-clip factor g (sl2).  Parity (pi) is folded into the
    # host-built E table so the program is core-independent.
    def attention(sl):
        causal = sl == 1
        banded = sl != 3
        e_sb = e1t_sb if sl == 1 else e2t_sb
        pcs = {}

        def bandpath(h):
            bsb = work.tile([128, 4, 640], BF, tag="bsb", bufs=2,
                            name=f"bsb{sl}_{h}")
            b0 = 64 * (h % 2)
            for l in range(4):
                for seg in range(2):
                    bp = psum.tile([128, 320], F32, tag="mm", bufs=3,
                                   name=f"bp{sl}_{h}_{l}_{seg}")
                    te.matmul(bp, lhsT=qth(h)[:, 128 * l:128 * (l + 1)],
                              rhs=e_sb[b0:b0 + 64, 320 * seg:320 * (seg + 1)],
                              start=True, stop=not causal)
                    if causal:
                        te.matmul(bp, lhsT=onesrow,
                                  rhs=e1m_sb[:, 320 * seg:320 * (seg + 1)],
                                  start=False, stop=True)
                    sc.activation(bsb[:, l, 320 * seg:320 * (seg + 1)], bp,
                                  AF.Exp)
            bd = dram.tile([128, 4 * 640], BF, tag="bd", name=f"bd{sl}_{h}")
            sy.dma_start(bd, bsb.rearrange("p l c -> p (l c)"))
            # strip[ii, l, 128*s + jj] = G[ii, 640*l + 127 + 128*s + jj - ii]
            strip = work.tile([128, 4, 512], BF, tag="strip", bufs=2,
                              name=f"strip{sl}_{h}")
            src = bass.AP(tensor=bd.tensor, offset=bd.offset + 127,
                          ap=[[4 * 640 - 1, 128], [640, 4], [1, 512]])
            sy.dma_start(strip, src)
            pcs[h] = strip

        def scorespath(h):
            pt = ppool.tile([128, 8, 512], BF, tag="pt", name=f"pt{sl}_{h}")
            for kb in range(8):
                sps = psum.tile([128, 512], F32, tag="s", bufs=2,
                                name=f"sT{sl}_{h}_{kb}")
                te.matmul(sps, lhsT=kth(h)[:, 128 * kb:128 * (kb + 1)],
                          rhs=qth(h), start=True, stop=True)
                sc.activation(pt[:, kb, :], sps, AF.Exp, scale=1.0 / SCALE)
            if banded:
                strip = pcs.pop(h)
                tp = psum.tile([128, 512], BF, tag="tp", bufs=1,
                               name=f"tp{sl}_{h}")
                for l in range(4):
                    slo = 1 if l == 0 else 0
                    shi = 3 if l == 3 else 4
                    for st in range(slo, shi):
                        kb = 2 * l - 1 + st
                        te.transpose(tp[:, 128 * st:128 * (st + 1)],
                                     strip[:, l, 128 * st:128 * (st + 1)],
                                     ident128)
                        ve.tensor_mul(pt[:, kb, 128 * l:128 * (l + 1)],
                                      pt[:, kb, 128 * l:128 * (l + 1)],
                                      tp[:, 128 * st:128 * (st + 1)])
                if causal:
                    for kb in range(3, 8):
                        nfr = (kb - 3) // 2 + 1
                        gp.memset(pt[:, kb, 0:128 * nfr], 0.0)
                else:
                    gps = psum.tile([1, 512], F32, tag="mm", bufs=3,
                                    name=f"gps{sl}_{h}")
                    te.matmul(gps, lhsT=e_sb[64 * (h % 2):64 * (h % 2) + 64, 639:640],
                              rhs=qth(h), start=True, stop=True)
                    gr = smalls.tile([1, 512], BF, tag="gr", name=f"gr{sl}_{h}")
                    sc.activation(gr, gps, AF.Exp)
                    gb = work.tile([128, 512], BF, tag="gb", bufs=2,
                                   name=f"gb{sl}_{h}")
                    gp.partition_broadcast(gb, gr)
                    for kb in range(3, 8):
                        nfr = (kb - 3) // 2 + 1
                        ve.tensor_mul(pt[:, kb, 0:128 * nfr],
                                      pt[:, kb, 0:128 * nfr], gb[:, 0:128 * nfr])
            pv = psum.tile([65, 512], F32, tag="pv", bufs=2, name=f"pv{sl}_{h}")
            for kb in range(8):
                te.matmul(pv, lhsT=v_all[:, h, kb, :], rhs=pt[:, kb, :],
                          start=(kb == 0), stop=(kb == 7))
            rz = smalls.tile([1, 512], F32, tag="rz", name=f"rz{sl}_{h}")
            ve.reciprocal(rz, pv[64:65, :])
            zb = work.tile([64, 512], F32, tag="zb", bufs=2, name=f"zb{sl}_{h}")
            gp.partition_broadcast(zb, rz)
            ve.tensor_mul(aT[64 * (h % 2):64 * (h % 2) + 64, h // 2, :],
                          pv[0:64, :], zb)

        if banded:
            for h in range(H + 2):
                if h < H:
                    bandpath(h)
                if h >= 2:
                    scorespath(h - 2)
        else:
            for h in range(H):
                scorespath(h)

    # ---------------- output proj + residual ----------------
    def o_proj(wname, bias0):
        def ep(ct, rc, ps):
            ve.scalar_tensor_tensor(xres[:, ct, :], ps, bias_ap(bias0 + ct),
                                    xres[:, ct, :], op0=AL.add, op1=AL.add)
        proj(wname, 0, 8, bias0, lambda kt, rc: aT[:, kt, :], 1, 8, ep)

    # ---------------- layernorm (feature-major) ----------------
    def layer_norm(gname, bname, final_out=None):
        s1 = psum.tile([1, 512], F32, tag="pv", bufs=2, name=f"lns1_{gname}_{1 if final_out is None else 2}")
        s2 = psum.tile([1, 512], F32, tag="pv", bufs=2, name=f"lns2_{gname}_{1 if final_out is None else 2}")
        for dt in range(8):
            bx = work.tile([128, 512], BF, tag="lnbx", bufs=3, name=f"lnbx_{gname}_{dt}")
            gp.tensor_copy(bx, xres[:, dt, :])
            sq = work.tile([128, 512], BF, tag="lnsq", bufs=3, name=f"lnsq_{gname}_{dt}")
            ve.tensor_mul(sq, xres[:, dt, :], xres[:, dt, :])
            te.matmul(s1, lhsT=ones_sb, rhs=bx,
                      start=(dt == 0), stop=(dt == 7))
            te.matmul(s2, lhsT=ones_sb, rhs=sq,
                      start=(dt == 0), stop=(dt == 7))
        mean = smalls.tile([1, 512], F32, tag="ln", name=f"lnmean_{gname}")
        ve.tensor_scalar_mul(mean, s1, 1.0 / D)
        rstd = smalls.tile([1, 512], F32, tag="ln", name=f"lnrstd_{gname}")
        ve.tensor_mul(rstd, mean, mean)                                  # mean^2
        ve.scalar_tensor_tensor(rstd, s2, 1.0 / D, rstd,
                                op0=AL.mult, op1=AL.subtract)            # var
        sc.activation(rstd, rstd, AF.Sqrt, bias=eps_sb)                  # sd
        ve.reciprocal(rstd, rstd)                                        # 1/sd
        mb = work.tile([128, 512], F32, tag="lnb", bufs=2, name=f"lnmb_{gname}")
        gp.partition_broadcast(mb, mean)
        rb = work.tile([128, 512], F32, tag="lnb", bufs=2, name=f"lnrb_{gname}")
        gp.partition_broadcast(rb, rstd)
        gcol, bcol = BIAS_COL[gname], BIAS_COL[bname]
        for dt in range(8):
            mgr = work.tile([128, 512], F32, tag="lg", bufs=3, name=f"lnmgr_{gname}_{dt}")
            ve.scalar_tensor_tensor(mgr, mb, bias_ap(gcol + dt), rb,
                                    op0=AL.mult, op1=AL.mult)
            cc = work.tile([128, 512], F32, tag="lg", bufs=3, name=f"lncc_{gname}_{dt}")
            ve.tensor_scalar(cc, mgr, -1.0, bias_ap(bcol + dt),
                             op0=AL.mult, op1=AL.add)
            t = work.tile([128, 512], F32, tag="lg", bufs=3, name=f"lnt_{gname}_{dt}")
            ve.scalar_tensor_tensor(t, xres[:, dt, :], bias_ap(gcol + dt), rb,
                                    op0=AL.mult, op1=AL.mult)
            if final_out is not None:
                ot = work.tile([128, 512], F32, tag="lg", bufs=3, name=f"lnot_{gname}_{dt}")
                ve.tensor_add(ot, t, cc)
                sy.dma_start(final_out[128 * dt:128 * dt + 128, :], ot)
            else:
                ve.tensor_add(xres[:, dt, :], t, cc)
                sc.activation(xbf[:, dt, :], xres[:, dt, :], AF.Copy)

    # ================= sublayer 1 =================
    load_enc("sfb")
    proj("wqkv", 0, 8, BIAS_COL["qkv"], rhs_own, 1, 8, q_ep(BIAS_COL["qkv"]))
    proj("wqkv", 8, 8, BIAS_COL["qkv"] + 8, rhs_enc, 2, 8,
         k_ep(BIAS_COL["qkv"] + 8, False))
    proj("wqkv", 16, 8, BIAS_COL["qkv"] + 16, rhs_enc, 2, 8,
         v_ep(BIAS_COL["qkv"] + 16))
    if debug:
        sy.dma_start(T["d_qt"], qt[0])
        sy.dma_start(T["d_kt"], kt_all[0:64, 0, :])
        sy.dma_start(T["d_v"], v_all[:, 0, :, :])
    attention(1)
    if debug:
        sy.dma_start(T["d_at"], aT)
    o_proj("wo1", BIAS_COL["o1"])
    layer_norm("ln1g", "ln1b")
    if debug:
        sy.dma_start(T["d_x1"], xres)

    # ================= sublayer 2 =================
    load_enc("chb")
    proj("wq2", 0, 8, BIAS_COL["q2"], rhs_own, 1, 8, q_ep(BIAS_COL["q2"]))
    proj("wk2", 0, 8, BIAS_COL["k2"], rhs_enc, 2, 8, k_ep(BIAS_COL["k2"], False))
    proj("wv2", 0, 8, BIAS_COL["v2"], rhs_enc, 2, 8, v_ep(BIAS_COL["v2"]))
    attention(2)
    o_proj("wo2", BIAS_COL["o2"])
    layer_norm("ln2g", "ln2b")

    # ================= sublayer 3 =================
    load_enc("wdb")
    proj("wq3", 0, 8, BIAS_COL["q3"], rhs_own, 1, 8, q_ep(BIAS_COL["q3"]))
    proj("wk3", 0, 8, BIAS_COL["k3"], rhs_enc, 2, 8, k_ep(BIAS_COL["k3"], True))
    proj("wv3", 0, 8, BIAS_COL["v3"], rhs_enc, 2, 8, v_ep(BIAS_COL["v3"]))
    attention(3)
    o_proj("wo3", BIAS_COL["o3"])
    layer_norm("ln3g", "ln3b")

    # ================= FFN =================
    def f1_ep(ct, rc, ps):
        sc.activation(h1[:, ct, :], ps, AF.Relu, bias=bias_ap(BIAS_COL["f1"] + ct))
    proj("wf1", 0, 32, BIAS_COL["f1"], rhs_own, 1, 8, f1_ep)

    def f2_ep(ct, rc, ps):
        ve.scalar_tensor_tensor(xres[:, ct, :], ps, bias_ap(BIAS_COL["f2"] + ct),
                                xres[:, ct, :], op0=AL.add, op1=AL.add)
    proj("wf2", 0, 8, BIAS_COL["f2"], lambda kt, rc: h1[:, kt, :], 1, 32, f2_ep)

    layer_norm("ln3g", "ln3b", final_out=T["yT"])


def build_nc(debug=False, reps=1):
    nc = bacc.Bacc("TRN2", target_bir_lowering=False, debug=False)
    T = {}

    def din(name, shape, dt=BF):
        T[name] = nc.dram_tensor(name, shape, dt, kind="ExternalInput").ap()

    din("xow", [D, S_OWN], F32)
    din("xob", [D, S_OWN])
    din("sfb", [D, LK])
    din("chb", [D, LK])
    din("wdb", [D, LK])
    din("wqkv", [24, 8, 128, 128])
    for w in ["wo1", "wq2", "wk2", "wv2", "wo2", "wq3", "wk3", "wv3", "wo3"]:
        din(w, [8, 8, 128, 128])
    din("wf1", [32, 8, 128, 128])
    din("wf2", [8, 32, 128, 128])
    din("bias", [128, NBIAS], F32)
    din("e1t", [128, WE])
    din("e2t", [128, WE])
    din("e1m", [1, WE])
    din("g3t", [128, LK])
    T["yT"] = nc.dram_tensor("yT", [D, S_OWN], F32, kind="ExternalOutput").ap()
    if debug:
        def dout(name, shape, dt=BF):
            T[name] = nc.dram_tensor(name, shape, dt, kind="ExternalOutput").ap()
        dout("d_qt", [65, S_OWN])
        dout("d_kt", [64, LK])
        dout("d_v", [128, 8, 65])
        dout("d_at", [128, 8, S_OWN])
        dout("d_x1", [128, 8, S_OWN], F32)
        dout("d_band", [128, WE])
        dout("d_strip", [128, 512])
        dout("d_lg", [128, 512], F32)
        dout("d_p", [128, 512])

    from contextlib import ExitStack
    with tile.TileContext(nc) as tc:
        for _ in range(reps):
            with ExitStack() as ctx:
                _emit(nc, tc, ctx, T, debug=debug)
    nc.compile()
    return nc


_NC = None


def _get_nc():
    global _NC
    if _NC is None:
        _NC = build_nc()
    return _NC


# ======================= host side =======================

def _own_rows(pi):
    return np.concatenate([np.arange(128 * (2 * l + pi), 128 * (2 * l + pi) + 128)
                           for l in range(4)])


def _tile_w(w):
    K, N = w.shape
    return np.ascontiguousarray(
        w.reshape(K // 128, 128, N // 128, 128).transpose(2, 0, 1, 3)
    ).astype(BF16)


def _stripe(v):
    """bias vector [n*128] -> [128, n] per-partition stripes (fp32)."""
    n = v.shape[0] // 128
    return np.ascontiguousarray(v.reshape(n, 128).T).astype(np.float32)


def _build_E(pos_scaled, pi, causal):
    """E table [65, 640]: col c <-> relpos r = c - 255 - 128*pi.  Rows 0-63:
    (pos[clip(r)] - pos[-M]) / SCALE (divide-through by the left-clip factor);
    row 64 (hit by qt's ones row): causal mask.  Col 639 doubles as the
    right-clip g column (r clips to +M there for both parities)."""
    c = np.arange(WE)
    r = c - 255 - 128 * pi
    idx = np.clip(r, -M, M) + M
    body = (pos_scaled[idx] - pos_scaled[0]).T.astype(np.float32)  # [64, WE]
    dup = np.concatenate([body, body], axis=0)                     # [128, WE]
    mask = np.where(r > 0, NEG, 0.0)[None, :].astype(np.float32) if causal \
        else np.zeros((1, WE), np.float32)
    return dup.astype(BF16), mask.astype(BF16)


def _qpos(sentence_lengths):
    s = np.asarray(sentence_lengths, np.int64)
    offsets = s - np.cumsum(s)
    B = int(s.sum())
    return np.repeat(offsets, s)[:B] + np.arange(B)


def _host_prep(inp):
    qkv_w = np.asarray(inp["qkv_w"], np.float32)
    wq = qkv_w.reshape(D, H, 3, DH)
    wqkv_r = np.concatenate([wq[:, :, 0], wq[:, :, 1], wq[:, :, 2]], axis=1)
    wqkv_r = wqkv_r.reshape(D, 3 * D)
    qb = np.asarray(inp["qkv_b"], np.float32).reshape(H, 3, DH)
    qkv_b_r = np.concatenate([qb[:, 0], qb[:, 1], qb[:, 2]], axis=0).reshape(3 * D)

    bias = np.zeros((128, NBIAS), np.float32)

    def put(name, vec):
        c = BIAS_COL[name]
        s = _stripe(np.asarray(vec, np.float32))
        bias[:, c:c + s.shape[1]] = s

    put("qkv", qkv_b_r)
    for n, k in [("o1", "o1_b"), ("q2", "q2_b"), ("k2", "k2_b"), ("v2", "v2_b"),
                 ("o2", "o2_b"), ("q3", "q3_b"), ("k3", "k3_b"), ("v3", "v3_b"),
                 ("o3", "o3_b"), ("f1", "f1_b"), ("f2", "f2_b"),
                 ("ln1g", "ln1_g"), ("ln1b", "ln1_b"), ("ln2g", "ln2_g"),
                 ("ln2b", "ln2_b"), ("ln3g", "ln3_g"), ("ln3b", "ln3_b")]:
        put(n, inp[k])

    weights = {
        "wqkv": _tile_w(wqkv_r),
        "wo1": _tile_w(np.asarray(inp["o1_w"], np.float32)),
        "wq2": _tile_w(np.asarray(inp["q2_w"], np.float32)),
        "wk2": _tile_w(np.asarray(inp["k2_w"], np.float32)),
        "wv2": _tile_w(np.asarray(inp["v2_w"], np.float32)),
        "wo2": _tile_w(np.asarray(inp["o2_w"], np.float32)),
        "wq3": _tile_w(np.asarray(inp["q3_w"], np.float32)),
        "wk3": _tile_w(np.asarray(inp["k3_w"], np.float32)),
        "wv3": _tile_w(np.asarray(inp["v3_w"], np.float32)),
        "wo3": _tile_w(np.asarray(inp["o3_w"], np.float32)),
        "wf1": _tile_w(np.asarray(inp["f1_w"], np.float32)),
        "wf2": _tile_w(np.asarray(inp["f2_w"], np.float32)),
        "bias": bias,
    }

    pos1s = np.asarray(inp["pos1"], np.float32) / SCALE
    pos2s = np.asarray(inp["pos2"], np.float32) / SCALE
    pos3 = np.asarray(inp["pos3"], np.float32)
    e1 = [_build_E(pos1s, pi, True) for pi in range(2)]
    e2 = [_build_E(pos2s, pi, False) for pi in range(2)]

    qpos = _qpos(inp["sentence_lengths"])
    g3 = []
    for b in range(4):
        idx = np.clip(np.arange(LK) - int(qpos[b]), -M, M) + M
        g = pos3[idx].T.astype(BF16)          # [64, LK]
        g3.append(np.concatenate([g, g], axis=0))  # [128, LK] duplicated

    x = np.asarray(inp["self_input"], np.float32)
    ch = np.asarray(inp["char_enc"], np.float32)
    wd = np.asarray(inp["word_enc"], np.float32)

    in_maps = []
    for core in range(8):
        b, pi = core // 2, core % 2
        rows = _own_rows(pi)
        xT = np.ascontiguousarray(x[b].T)            # [D, 1024]
        m = dict(weights)
        m["xow"] = np.ascontiguousarray(xT[:, rows])
        m["xob"] = m["xow"].astype(BF16)
        m["sfb"] = xT.astype(BF16)
        m["chb"] = np.ascontiguousarray(ch[b].T).astype(BF16)
        m["wdb"] = np.ascontiguousarray(wd[b].T).astype(BF16)
        m["e1t"] = e1[pi][0]
        m["e1m"] = e1[pi][1]
        m["e2t"] = e2[pi][0]
        m["g3t"] = g3[b]
        in_maps.append(m)
    return in_maps


def _fast_path_ok(inp):
    lam = np.asarray(inp["look_ahead_mask"])
    B, Lq = 4, 1024
    if lam.shape != (1, 1, Lq, Lq):
        return False
    causal = np.triu(np.ones((Lq, Lq), bool), k=1)
    if not np.array_equal(lam[0, 0].astype(bool), causal):
        return False
    if np.asarray(inp["char_mask"]).any() or np.asarray(inp["word_mask"]).any():
        return False
    if np.asarray(inp["sentence_lengths"]).sum() != B:
        return False
    return True


def _numpy_reference(inp):
    """Pure-numpy fallback (slow but exact) for unexpected mask patterns."""
    f = lambda k: np.asarray(inp[k], np.float32)

    def ln(x, g, b):
        m = x.mean(-1, keepdims=True)
        v = ((x - m) ** 2).mean(-1, keepdims=True)
        return (x - m) / np.sqrt(v + EPS) * g + b

    def split_heads(x):
        B, S, _ = x.shape
        return x.reshape(B, S, H, DH).transpose(0, 2, 1, 3)

    def softmax(x):
        x = x - x.max(-1, keepdims=True)
        e = np.exp(x)
        return e / e.sum(-1, keepdims=True)

    def attn(Q, K, V, pl, mask):
        logits = (np.einsum('bhid,bhjd->bhij', Q, K) + pl) / SCALE
        logits = np.where(mask, -np.inf, logits)
        p = softmax(logits)
        out = np.einsum('bhij,bhjd->bhid', p, V)
        B, h, S, dh = out.shape
        return out.transpose(0, 2, 1, 3).reshape(B, S, h * dh)

    def char_pos(emb, lq, lk):
        idx = np.clip(np.arange(lk)[None, :] - np.arange(lq)[:, None], -M, M) + M
        return emb[idx]

    x0 = f("self_input")
    B, Lq, _ = x0.shape
    qkv = (x0 @ f("qkv_w") + f("qkv_b")).reshape(B, Lq, H, 3 * DH).transpose(0, 2, 1, 3)
    Q, K, V = np.split(qkv, 3, axis=-1)
    pl = np.einsum('bhid,ijd->bhij', Q, char_pos(f("pos1"), Lq, Lq))
    a = attn(Q, K, V, pl, np.asarray(inp["look_ahead_mask"])) @ f("o1_w") + f("o1_b")
    x = ln(a + x0, f("ln1_g"), f("ln1_b"))

    ce = f("char_enc")
    Q = split_heads(x @ f("q2_w") + f("q2_b"))
    K = split_heads(ce @ f("k2_w") + f("k2_b"))
    V = split_heads(ce @ f("v2_w") + f("v2_b"))
    pl = np.einsum('bhid,ijd->bhij', Q, char_pos(f("pos2"), Lq, ce.shape[1]))
    a = attn(Q, K, V, pl, np.asarray(inp["char_mask"])) @ f("o2_w") + f("o2_b")
    x = ln(a + x, f("ln2_g"), f("ln2_b"))

    we = f("word_enc")
    Q = split_heads(x @ f("q3_w") + f("q3_b"))
    K = split_heads(we @ f("k3_w") + f("k3_b"))
    V = split_heads(we @ f("v3_w") + f("v3_b"))
    qpos = _qpos(inp["sentence_lengths"])
    idx = np.clip(np.arange(we.shape[1])[None, :] - qpos[:, None], -M, M) + M
    pl = np.einsum('bhid,bjd->bhij', Q, f("pos3")[idx])
    a = attn(Q, K, V, pl, np.asarray(inp["word_mask"])) @ f("o3_w") + f("o3_b")
    x = ln(a + x, f("ln3_g"), f("ln3_b"))

    ffn = np.maximum(x @ f("f1_w") + f("f1_b"), 0.0) @ f("f2_w") + f("f2_b")
    return ln(ffn + x, f("ln3_g"), f("ln3_b"))


def kernel(**inputs) -> np.ndarray:
    if not _fast_path_ok(inputs):
        return _numpy_reference(inputs)
    nc = _get_nc()
    in_maps = _host_prep(inputs)
    res = run_bass_kernel_spmd(nc, in_maps, list(range(8)))
    y = np.empty((4, 1024, 1024), np.float32)
    for core in range(8):
        b, pi = core // 2, core % 2
        yT = res.results[core]["yT"]
        y[b, _own_rows(pi), :] = yT.T
    return y



# revision 30
# speedup vs baseline: 1.2677x; 1.2677x over previous
"""Trainium2 Bass kernel for nn_DecoderLayer_84404697301735.

3-sublayer decoder (self-attn w/ char rel-pos, cross-attn to char encoder
w/ rel-pos, cross-attn to word encoder w/ word-level pos) + FFN.

Sharding: 8 cores = 4 batch x 2 interleaved query-tile halves.  Each core
computes 512 query rows end-to-end (feature-major layout); K/V projections
over the full 1024 keys are duplicated within a batch pair.  No collectives.

Relative-position logits: Band[i,t] = Q[i] . E[t] via matmul against a
host-built extended pos table E (clip folded in, causal mask folded into a
65th ones-row of Q), then a skewed-stride DMA read from a DRAM round-trip
converts band (query-relative) layout to absolute key layout.  The per-core
query-offset parity is folded into E so the program is core-independent.
"""

import numpy as np
import ml_dtypes

import concourse.bass as bass
import concourse.tile as tile
from concourse import bacc, mybir
from concourse.bass_utils import run_bass_kernel_spmd

BF16 = ml_dtypes.bfloat16
F32 = mybir.dt.float32
F32R = mybir.dt.float32r
BF = mybir.dt.bfloat16

D = 1024
H = 16
DH = 64
S_OWN = 512          # own query rows per core
LK = 1024            # keys
DFF = 4096
M = 128              # pos clip radius
WE = 640             # extended pos table width (r = c - 255 - 128*pi)
SCALE = float(DH) ** 0.5   # 8.0
EPS = 1e-5
NEG = -1e30

AL = mybir.AluOpType
AF = mybir.ActivationFunctionType

# bias_cat column layout (each unit = 1 col of [128, n] per-partition stripes)
_BIAS_SECTS = [
    ("qkv", 24), ("o1", 8), ("q2", 8), ("k2", 8), ("v2", 8), ("o2", 8),
    ("q3", 8), ("k3", 8), ("v3", 8), ("o3", 8), ("f1", 32), ("f2", 8),
    ("ln1g", 8), ("ln1b", 8), ("ln2g", 8), ("ln2b", 8), ("ln3g", 8), ("ln3b", 8),
]
BIAS_COL = {}
_c = 0
for _n, _w in _BIAS_SECTS:
    BIAS_COL[_n] = _c
    _c += _w
NBIAS = _c  # 184


def _emit(nc, tc, ctx, T, debug=False):
    """Emit the whole per-core program.  T: dict name -> dram AP."""
    te, ve, sc, gp, sy = nc.tensor, nc.vector, nc.scalar, nc.gpsimd, nc.sync

    singles = ctx.enter_context(tc.tile_pool(name="singles", bufs=1))
    psum = ctx.enter_context(tc.tile_pool(name="psum", bufs=1, space="PSUM"))
    wpool = ctx.enter_context(tc.tile_pool(name="wpool", bufs=2))
    work = ctx.enter_context(tc.tile_pool(name="work", bufs=3))
    ppool = ctx.enter_context(tc.tile_pool(name="ppool", bufs=2))
    dram = ctx.enter_context(tc.tile_pool(name="dramp", bufs=4, space="DRAM"))
    smalls = ctx.enter_context(tc.tile_pool(name="smalls", bufs=2))
    # PSUM budget (8 banks): mm=3, s=2, pv=2, tp=1

    # ---- persistent SBUF ----
    bias_sb = singles.tile([128, NBIAS], F32)
    sy.dma_start(bias_sb, T["bias"])
    e1t_sb = singles.tile([128, WE], BF)
    sy.dma_start(e1t_sb, T["e1t"])
    e2t_sb = singles.tile([128, WE], BF)
    sy.dma_start(e2t_sb, T["e2t"])
    e1m_sb = singles.tile([1, WE], BF)
    sy.dma_start(e1m_sb, T["e1m"])
    g3t_sb = singles.tile([128, LK], BF)
    sy.dma_start(g3t_sb, T["g3t"])
    eps_sb = singles.tile([1, 1], F32)
    ve.memset(eps_sb, EPS)
    ones_sb = singles.tile([128, 1], BF)
    ve.memset(ones_sb, 1.0)

    xres = singles.tile([128, 8, S_OWN], F32)      # residual stream (feature-major)
    sy.dma_start(xres, T["xow"].rearrange("(a p) r -> p a r", p=128))
    xbf = singles.tile([128, 8, S_OWN], BF)        # bf16 copy for proj rhs
    sy.dma_start(xbf, T["xob"].rearrange("(a p) r -> p a r", p=128))
    big = singles.tile([128, 32, S_OWN], BF)       # enc (sublayers) / FFN hidden
    enc = big.rearrange("p a r -> p (a r)")[:, 0:8 * LK].rearrange(
        "p (a r) -> p a r", a=8)                   # current sublayer's enc input
    kt_all = singles.tile([128, 8, LK], BF)        # K^T, head h at rows 64*(h%2)

    def kth(h):
        return kt_all[64 * (h % 2):64 * (h % 2) + 64, h // 2, :]
    v_all = singles.tile([128, H, 8, 65], BF)      # V key-major + ones col
    gp.memset(v_all, 1.0)
    qt_pk = singles.tile([128, 8, S_OWN], BF)      # Q^T, head h at rows 64*(h%2)

    def qth(h):
        return qt_pk[64 * (h % 2):64 * (h % 2) + 64, h // 2, :]

    onesrow = singles.tile([1, 128], BF)
    gp.memset(onesrow, 1.0)
    aT = singles.tile([128, 8, S_OWN], BF)         # attention output (feature-major)
    h1 = big                                       # FFN hidden (aliases enc)
    ident64 = singles.tile([64, 64], BF)
    ident128 = singles.tile([128, 128], BF)
    from concourse.masks import make_identity
    make_identity(nc, ident64)
    make_identity(nc, ident128)

    def load_enc(name):
        sy.dma_start(enc, T[name].rearrange("(a p) r -> p a r", p=128))

    def bias_ap(col, base=0, size=128):
        return bias_sb[base:base + size, col:col + 1]

    # ---------------- projections ----------------
    # weights in DRAM are ct-major: [nct, nkt, 128, 128]; load a group of
    # ct columns in one DMA strip to amortize per-DMA issue cost.
    def proj(wname, ct0, nct, bias0, rhs_fn, nrc, nkt, epilogue):
        wd = T[wname]
        cchunk = max(1, 16 // nkt)  # ~2048 cols per strip
        for cg in range(0, nct, cchunk):
            ncg = min(cchunk, nct - cg)
            ws = wpool.tile([128, ncg * nkt * 128], BF, tag="w",
                            name=f"ws_{wname}_{ct0}_{cg}")
            src = wd[ct0 + cg:ct0 + cg + ncg].rearrange("a k p c -> p (a k) c")
            sy.dma_start(ws.rearrange("p (a c) -> p a c", c=128), src)
            for ci in range(ncg):
                ct = cg + ci
                pss = []
                for rc in range(nrc):
                    ps = psum.tile([128, 512], F32, tag="mm", bufs=2,
                                   name=f"ps_{wname}_{ct0}_{ct}_{rc}")
                    pss.append(ps)
                for kt in range(nkt):
                    wt = ws[:, (ci * nkt + kt) * 128:(ci * nkt + kt + 1) * 128]
                    for rc in range(nrc):
                        te.matmul(pss[rc], lhsT=wt, rhs=rhs_fn(kt, rc),
                                  start=(kt == 0), stop=(kt == nkt - 1))
                for rc in range(nrc):
                    epilogue(ct, rc, pss[rc])

    def rhs_own(kt, rc):
        return xbf[:, kt, :]

    def rhs_enc_rc(rc):
        def f(kt, _):
            return enc[:, kt, 512 * rc:512 * rc + 512]
        return f

    def q_ep(bias0):
        def ep(ct, rc, ps):
            for sub in range(2):
                h = 2 * ct + sub
                sc.activation(qth(h), ps[64 * sub:64 * sub + 64, :], AF.Identity,
                              bias=bias_ap(bias0 + ct, 64 * sub, 64))
        return ep

    def k_ep(bias0, add_g, rc):
        def ep(ct, _, ps):
            for sub in range(2):
                h = 2 * ct + sub
                dst = kth(h)[:, 512 * rc:512 * rc + 512]
                src = ps[64 * sub:64 * sub + 64, :]
                b = bias_ap(bias0 + ct, 64 * sub, 64)
                if add_g:
                    ve.scalar_tensor_tensor(
                        dst, src, b, g3t_sb[64 * sub:64 * sub + 64, 512 * rc:512 * rc + 512],
                        op0=AL.add, op1=AL.add)
                else:
                    ve.tensor_scalar_add(dst, src, b)
        return ep

    def v_ep(bias0, rc):
        def ep(ct, _, ps):
            for sub in range(2):
                h = 2 * ct + sub
                vt = work.tile([64, 512], BF, tag="vt", name=f"vt_{h}_{rc}")
                sc.activation(vt, ps[64 * sub:64 * sub + 64, :], AF.Identity,
                              bias=bias_ap(bias0 + ct, 64 * sub, 64))
                for j in range(4):
                    tp = psum.tile([128, 64], BF, tag="x", bufs=2,
                                   name=f"vtp_{h}_{rc}_{j}")
                    te.transpose(tp, vt[:, 128 * j:128 * j + 128], ident64)
                    ve.tensor_copy(v_all[:, h, 4 * rc + j, 0:64], tp)
        return ep

    # ---------------- attention (transposed scores) ----------------
    # pT[kb] = exp((K_kb Q^T)/s) [keys, queries]; PV consumes pT directly so
    # P is never transposed.  Relative-position factors via the identity
    # exp(S+B) = exp(S)*exp(B), divided through per query by the left-clip
    # factor: fully-left key tiles need no fixup; a 4-key-tile window around
    # the diagonal is recomputed in [q,k] orientation with the band added via
    # one batched skew-DMA per head, exp'd, and PE-transposed into pT;
    # fully-right tiles are memset to 0 (causal sl1) or scaled by the
    # per-query right-clip factor g (sl2).  Parity (pi) is folded into the
    # host-built E table so the program is core-independent.
    def attention(sl, emit_kv=None):
        causal = sl == 1
        banded = sl != 3
        e_sb = e1t_sb if sl == 1 else e2t_sb
        pcs = {}
        pts = {}

        def bandpath(h):
            bsb = work.tile([128, 4, 640], BF, tag="bsb", bufs=2,
                            name=f"bsb{sl}_{h}")
            bsbf = bsb.rearrange("p l c -> p (l c)")
            b0 = 64 * (h % 2)
            # 2560 band cols in 3 psum tiles; matmul segs split at bank AND
            # l-block boundaries (lhsT changes per l, dest must stay in-bank)
            for t0, t1 in [(0, 1024), (1024, 2048), (2048, 2560)]:
                bt = psum.tile([128, t1 - t0], F32, tag="s", bufs=2,
                               name=f"bt{sl}_{h}_{t0}")
                cuts = sorted({t0, t1}
                              | {c for c in (512, 1024, 1536, 2048) if t0 < c < t1}
                              | {c for c in (640, 1280, 1920) if t0 < c < t1})
                for gc0, gc1 in zip(cuts[:-1], cuts[1:]):
                    l = gc0 // 640
                    ec0, ec1 = gc0 - 640 * l, gc1 - 640 * l
                    te.matmul(bt[:, gc0 - t0:gc1 - t0],
                              lhsT=qth(h)[:, 128 * l:128 * (l + 1)],
                              rhs=e_sb[b0:b0 + 64, ec0:ec1],
                              start=True, stop=not causal)
                    if causal:
                        te.matmul(bt[:, gc0 - t0:gc1 - t0], lhsT=onesrow,
                                  rhs=e1m_sb[:, ec0:ec1], start=False, stop=True)
                sc.activation(bsbf[:, t0:t1], bt, AF.Exp)
            bd = dram.tile([128, 4 * 640], BF, tag="bd", name=f"bd{sl}_{h}")
            gp.dma_start(bd, bsb.rearrange("p l c -> p (l c)"))
            # strip[ii, l, 128*s + jj] = G[ii, 640*l + 127 + 128*s + jj - ii]
            strip = work.tile([128, 4, 512], BF, tag="strip", bufs=3,
                              name=f"strip{sl}_{h}")
            src = bass.AP(tensor=bd.tensor, offset=bd.offset + 127,
                          ap=[[4 * 640 - 1, 128], [640, 4], [1, 512]])
            gp.dma_start(strip, src)
            pcs[h] = strip

        def st_part(h):
            pt = ppool.tile([128, 8, 512], BF, tag="pt", name=f"pt{sl}_{h}")
            for kp in range(4):
                sps = psum.tile([128, 1024], F32, tag="s", bufs=2,
                                name=f"sT{sl}_{h}_{kp}")
                for j in range(2):
                    kb = 2 * kp + j
                    te.matmul(sps[:, 512 * j:512 * (j + 1)],
                              lhsT=kth(h)[:, 128 * kb:128 * (kb + 1)],
                              rhs=qth(h), start=True, stop=True)
                if causal:
                    # skip exp on columns that will be memset to zero anyway
                    for j in range(2):
                        kb = 2 * kp + j
                        c0 = 128 * ((kb - 3) // 2 + 1) if kb >= 3 else 0
                        sc.activation(pt[:, kb, c0:], sps[:, 512 * j + c0:512 * (j + 1)],
                                      AF.Exp, scale=1.0 / SCALE)
                else:
                    dst = pt.rearrange("p a r -> p (a r)")[:, 1024 * kp:1024 * (kp + 1)]
                    sc.activation(dst, sps, AF.Exp, scale=1.0 / SCALE)
            pts[h] = pt

        def fix_part(h):
            pt = pts.pop(h)
            if banded:
                strip = pcs.pop(h)
                for l in range(4):
                    slo = 1 if l == 0 else 0
                    shi = 3 if l == 3 else 4
                    nsl = shi - slo
                    kb0 = 2 * l - 1 + slo
                    tp = psum.tile([128, 512], BF, tag="x", bufs=2,
                                   name=f"tp{sl}_{h}_{l}")
                    for j in range(nsl):
                        te.transpose(tp[:, 128 * j:128 * (j + 1)],
                                     strip[:, l, 128 * (slo + j):128 * (slo + j + 1)],
                                     ident128)
                    dst = pt[:, kb0:kb0 + nsl, 128 * l:128 * (l + 1)]
                    ve.tensor_mul(dst, dst,
                                  tp[:, 0:128 * nsl].rearrange(
                                      "p (a c) -> p a c", c=128))
                if causal:
                    for kb in range(3, 8):
                        nfr = (kb - 3) // 2 + 1
                        gp.memset(pt[:, kb, 0:128 * nfr], 0.0)
                else:
                    gps = psum.tile([1, 512], F32, tag="mm", bufs=2,
                                    name=f"gps{sl}_{h}")
                    te.matmul(gps, lhsT=e_sb[64 * (h % 2):64 * (h % 2) + 64, 639:640],
                              rhs=qth(h), start=True, stop=True)
                    gr = smalls.tile([1, 512], BF, tag="gr", name=f"gr{sl}_{h}")
                    sc.activation(gr, gps, AF.Exp)
                    gb = work.tile([128, 512], BF, tag="gb", bufs=2,
                                   name=f"gb{sl}_{h}")
                    gp.partition_broadcast(gb, gr)
                    for kb in range(3, 8):
                        nfr = (kb - 3) // 2 + 1
                        ve.tensor_mul(pt[:, kb, 0:128 * nfr],
                                      pt[:, kb, 0:128 * nfr], gb[:, 0:128 * nfr])
            pv = psum.tile([65, 512], F32, tag="x", bufs=2, name=f"pv{sl}_{h}")
            for kb in range(8):
                te.matmul(pv, lhsT=v_all[:, h, kb, :], rhs=pt[:, kb, :],
                          start=(kb == 0), stop=(kb == 7))
            rz = smalls.tile([1, 512], F32, tag="rz", name=f"rz{sl}_{h}")
            ve.reciprocal(rz, pv[64:65, :])
            zb = work.tile([64, 512], F32, tag="zb", bufs=2, name=f"zb{sl}_{h}")
            gp.partition_broadcast(zb, rz)
            ve.tensor_mul(aT[64 * (h % 2):64 * (h % 2) + 64, h // 2, :],
                          pv[0:64, :], zb)

        if banded:
            for h in range(3):
                bandpath(h)
            if emit_kv is not None:
                emit_kv()
            st_part(0)
            for h in range(H):
                if h + 3 < H:
                    bandpath(h + 3)
                if h + 1 < H:
                    st_part(h + 1)
                fix_part(h)
        else:
            if emit_kv is not None:
                emit_kv()
            st_part(0)
            for h in range(H):
                if h + 1 < H:
                    st_part(h + 1)
                fix_part(h)

    # ---------------- output proj + residual ----------------
    def o_proj(wname, bias0):
        def ep(ct, rc, ps):
            ve.scalar_tensor_tensor(xres[:, ct, :], ps, bias_ap(bias0 + ct),
                                    xres[:, ct, :], op0=AL.add, op1=AL.add)
        proj(wname, 0, 8, bias0, lambda kt, rc: aT[:, kt, :], 1, 8, ep)

    # ---------------- layernorm (feature-major) ----------------
    def layer_norm(gname, bname, final_out=None):
        s1 = psum.tile([1, 512], F32, tag="x", bufs=2, name=f"lns1_{gname}_{1 if final_out is None else 2}")
        s2 = psum.tile([1, 512], F32, tag="x", bufs=2, name=f"lns2_{gname}_{1 if final_out is None else 2}")
        for dt in range(8):
            bx = work.tile([128, 512], BF, tag="lnbx", bufs=3, name=f"lnbx_{gname}_{dt}")
            gp.tensor_copy(bx, xres[:, dt, :])
            sq = work.tile([128, 512], BF, tag="lnsq", bufs=3, name=f"lnsq_{gname}_{dt}")
            ve.tensor_mul(sq, xres[:, dt, :], xres[:, dt, :])
            te.matmul(s1, lhsT=ones_sb, rhs=bx,
                      start=(dt == 0), stop=(dt == 7))
            te.matmul(s2, lhsT=ones_sb, rhs=sq,
                      start=(dt == 0), stop=(dt == 7))
        mean = smalls.tile([1, 512], F32, tag="ln", name=f"lnmean_{gname}")
        ve.tensor_scalar_mul(mean, s1, 1.0 / D)
        rstd = smalls.tile([1, 512], F32, tag="ln", name=f"lnrstd_{gname}")
        ve.tensor_mul(rstd, mean, mean)                                  # mean^2
        ve.scalar_tensor_tensor(rstd, s2, 1.0 / D, rstd,
                                op0=AL.mult, op1=AL.subtract)            # var
        sc.activation(rstd, rstd, AF.Sqrt, bias=eps_sb)                  # sd
        ve.reciprocal(rstd, rstd)                                        # 1/sd
        mb = work.tile([128, 512], F32, tag="lnb", bufs=2, name=f"lnmb_{gname}")
        gp.partition_broadcast(mb, mean)
        rb = work.tile([128, 512], F32, tag="lnb", bufs=2, name=f"lnrb_{gname}")
        gp.partition_broadcast(rb, rstd)
        gcol, bcol = BIAS_COL[gname], BIAS_COL[bname]
        for dt in range(8):
            mgr = work.tile([128, 512], F32, tag="lg", bufs=3, name=f"lnmgr_{gname}_{dt}")
            ve.scalar_tensor_tensor(mgr, mb, bias_ap(gcol + dt), rb,
                                    op0=AL.mult, op1=AL.mult)
            cc = work.tile([128, 512], F32, tag="lg", bufs=3, name=f"lncc_{gname}_{dt}")
            ve.tensor_scalar(cc, mgr, -1.0, bias_ap(bcol + dt),
                             op0=AL.mult, op1=AL.add)
            t = work.tile([128, 512], F32, tag="lg", bufs=3, name=f"lnt_{gname}_{dt}")
            ve.scalar_tensor_tensor(t, xres[:, dt, :], bias_ap(gcol + dt), rb,
                                    op0=AL.mult, op1=AL.mult)
            if final_out is not None:
                ot = work.tile([128, 512], F32, tag="lg", bufs=3, name=f"lnot_{gname}_{dt}")
                ve.tensor_add(ot, t, cc)
                sy.dma_start(final_out[128 * dt:128 * dt + 128, :], ot)
            else:
                ve.tensor_add(xres[:, dt, :], t, cc)
                sc.activation(xbf[:, dt, :], xres[:, dt, :], AF.Copy)

    # ================= sublayer 1 =================
    load_enc("sfb")
    proj("wqkv", 0, 8, BIAS_COL["qkv"], rhs_own, 1, 8, q_ep(BIAS_COL["qkv"]))
    for rc in range(2):
        proj("wqkv", 8, 8, BIAS_COL["qkv"] + 8, rhs_enc_rc(rc), 1, 8,
             k_ep(BIAS_COL["qkv"] + 8, False, rc))
    for rc in range(2):
        proj("wqkv", 16, 8, BIAS_COL["qkv"] + 16, rhs_enc_rc(rc), 1, 8,
             v_ep(BIAS_COL["qkv"] + 16, rc))
    if debug:
        sy.dma_start(T["d_qt"], qt[0])
        sy.dma_start(T["d_kt"], kt_all[0:64, 0, :])
        sy.dma_start(T["d_v"], v_all[:, 0, :, :])
    attention(1)
    if debug:
        sy.dma_start(T["d_at"], aT)
    o_proj("wo1", BIAS_COL["o1"])
    layer_norm("ln1g", "ln1b")
    if debug:
        sy.dma_start(T["d_x1"], xres)

    # ================= sublayer 2 =================
    load_enc("chb")
    proj("wq2", 0, 8, BIAS_COL["q2"], rhs_own, 1, 8, q_ep(BIAS_COL["q2"]))
    def kv2():
        for rc in range(2):
            proj("wk2", 0, 8, BIAS_COL["k2"], rhs_enc_rc(rc), 1, 8,
                 k_ep(BIAS_COL["k2"], False, rc))
        for rc in range(2):
            proj("wv2", 0, 8, BIAS_COL["v2"], rhs_enc_rc(rc), 1, 8,
                 v_ep(BIAS_COL["v2"], rc))
    attention(2, kv2)
    o_proj("wo2", BIAS_COL["o2"])
    layer_norm("ln2g", "ln2b")

    # ================= sublayer 3 =================
    load_enc("wdb")
    proj("wq3", 0, 8, BIAS_COL["q3"], rhs_own, 1, 8, q_ep(BIAS_COL["q3"]))
    def kv3():
        for rc in range(2):
            proj("wk3", 0, 8, BIAS_COL["k3"], rhs_enc_rc(rc), 1, 8,
                 k_ep(BIAS_COL["k3"], True, rc))
        for rc in range(2):
            proj("wv3", 0, 8, BIAS_COL["v3"], rhs_enc_rc(rc), 1, 8,
                 v_ep(BIAS_COL["v3"], rc))
    attention(3, kv3)
    o_proj("wo3", BIAS_COL["o3"])
    layer_norm("ln3g", "ln3b")

    # ================= FFN =================
    def f1_ep(ct, rc, ps):
        sc.activation(h1[:, ct, :], ps, AF.Relu, bias=bias_ap(BIAS_COL["f1"] + ct))
    proj("wf1", 0, 32, BIAS_COL["f1"], rhs_own, 1, 8, f1_ep)

    def f2_ep(ct, rc, ps):
        ve.scalar_tensor_tensor(xres[:, ct, :], ps, bias_ap(BIAS_COL["f2"] + ct),
                                xres[:, ct, :], op0=AL.add, op1=AL.add)
    proj("wf2", 0, 8, BIAS_COL["f2"], lambda kt, rc: h1[:, kt, :], 1, 32, f2_ep)

    layer_norm("ln3g", "ln3b", final_out=T["yT"])


def build_nc(debug=False, reps=1):
    nc = bacc.Bacc("TRN2", target_bir_lowering=False, debug=False)
    T = {}

    def din(name, shape, dt=BF):
        T[name] = nc.dram_tensor(name, shape, dt, kind="ExternalInput").ap()

    din("xow", [D, S_OWN], F32)
    din("xob", [D, S_OWN])
    din("sfb", [D, LK])
    din("chb", [D, LK])
    din("wdb", [D, LK])
    din("wqkv", [24, 8, 128, 128])
    for w in ["wo1", "wq2", "wk2", "wv2", "wo2", "wq3", "wk3", "wv3", "wo3"]:
        din(w, [8, 8, 128, 128])
    din("wf1", [32, 8, 128, 128])
    din("wf2", [8, 32, 128, 128])
    din("bias", [128, NBIAS], F32)
    din("e1t", [128, WE])
    din("e2t", [128, WE])
    din("e1m", [1, WE])
    din("g3t", [128, LK])
    T["yT"] = nc.dram_tensor("yT", [D, S_OWN], F32, kind="ExternalOutput").ap()
    if debug:
        def dout(name, shape, dt=BF):
            T[name] = nc.dram_tensor(name, shape, dt, kind="ExternalOutput").ap()
        dout("d_qt", [65, S_OWN])
        dout("d_kt", [64, LK])
        dout("d_v", [128, 8, 65])
        dout("d_at", [128, 8, S_OWN])
        dout("d_x1", [128, 8, S_OWN], F32)
        dout("d_band", [128, WE])
        dout("d_strip", [128, 512])
        dout("d_lg", [128, 512], F32)
        dout("d_p", [128, 512])

    from contextlib import ExitStack
    with tile.TileContext(nc) as tc:
        for _ in range(reps):
            with ExitStack() as ctx:
                _emit(nc, tc, ctx, T, debug=debug)
    nc.compile()
    return nc


_NC = None


def _get_nc():
    global _NC
    if _NC is None:
        _NC = build_nc()
    return _NC


# ======================= host side =======================

def _own_rows(pi):
    return np.concatenate([np.arange(128 * (2 * l + pi), 128 * (2 * l + pi) + 128)
                           for l in range(4)])


def _tile_w(w):
    K, N = w.shape
    return np.ascontiguousarray(
        w.reshape(K // 128, 128, N // 128, 128).transpose(2, 0, 1, 3)
    ).astype(BF16)


def _stripe(v):
    """bias vector [n*128] -> [128, n] per-partition stripes (fp32)."""
    n = v.shape[0] // 128
    return np.ascontiguousarray(v.reshape(n, 128).T).astype(np.float32)


def _build_E(pos_scaled, pi, causal):
    """E table [65, 640]: col c <-> relpos r = c - 255 - 128*pi.  Rows 0-63:
    (pos[clip(r)] - pos[-M]) / SCALE (divide-through by the left-clip factor);
    row 64 (hit by qt's ones row): causal mask.  Col 639 doubles as the
    right-clip g column (r clips to +M there for both parities)."""
    c = np.arange(WE)
    r = c - 255 - 128 * pi
    idx = np.clip(r, -M, M) + M
    body = (pos_scaled[idx] - pos_scaled[0]).T.astype(np.float32)  # [64, WE]
    dup = np.concatenate([body, body], axis=0)                     # [128, WE]
    mask = np.where(r > 0, NEG, 0.0)[None, :].astype(np.float32) if causal \
        else np.zeros((1, WE), np.float32)
    return dup.astype(BF16), mask.astype(BF16)


def _qpos(sentence_lengths):
    s = np.asarray(sentence_lengths, np.int64)
    offsets = s - np.cumsum(s)
    B = int(s.sum())
    return np.repeat(offsets, s)[:B] + np.arange(B)


def _host_prep(inp):
    qkv_w = np.asarray(inp["qkv_w"], np.float32)
    wq = qkv_w.reshape(D, H, 3, DH)
    wqkv_r = np.concatenate([wq[:, :, 0], wq[:, :, 1], wq[:, :, 2]], axis=1)
    wqkv_r = wqkv_r.reshape(D, 3 * D)
    qb = np.asarray(inp["qkv_b"], np.float32).reshape(H, 3, DH)
    qkv_b_r = np.concatenate([qb[:, 0], qb[:, 1], qb[:, 2]], axis=0).reshape(3 * D)

    bias = np.zeros((128, NBIAS), np.float32)

    def put(name, vec):
        c = BIAS_COL[name]
        s = _stripe(np.asarray(vec, np.float32))
        bias[:, c:c + s.shape[1]] = s

    put("qkv", qkv_b_r)
    for n, k in [("o1", "o1_b"), ("q2", "q2_b"), ("k2", "k2_b"), ("v2", "v2_b"),
                 ("o2", "o2_b"), ("q3", "q3_b"), ("k3", "k3_b"), ("v3", "v3_b"),
                 ("o3", "o3_b"), ("f1", "f1_b"), ("f2", "f2_b"),
                 ("ln1g", "ln1_g"), ("ln1b", "ln1_b"), ("ln2g", "ln2_g"),
                 ("ln2b", "ln2_b"), ("ln3g", "ln3_g"), ("ln3b", "ln3_b")]:
        put(n, inp[k])

    weights = {
        "wqkv": _tile_w(wqkv_r),
        "wo1": _tile_w(np.asarray(inp["o1_w"], np.float32)),
        "wq2": _tile_w(np.asarray(inp["q2_w"], np.float32)),
        "wk2": _tile_w(np.asarray(inp["k2_w"], np.float32)),
        "wv2": _tile_w(np.asarray(inp["v2_w"], np.float32)),
        "wo2": _tile_w(np.asarray(inp["o2_w"], np.float32)),
        "wq3": _tile_w(np.asarray(inp["q3_w"], np.float32)),
        "wk3": _tile_w(np.asarray(inp["k3_w"], np.float32)),
        "wv3": _tile_w(np.asarray(inp["v3_w"], np.float32)),
        "wo3": _tile_w(np.asarray(inp["o3_w"], np.float32)),
        "wf1": _tile_w(np.asarray(inp["f1_w"], np.float32)),
        "wf2": _tile_w(np.asarray(inp["f2_w"], np.float32)),
        "bias": bias,
    }

    pos1s = np.asarray(inp["pos1"], np.float32) / SCALE
    pos2s = np.asarray(inp["pos2"], np.float32) / SCALE
    pos3 = np.asarray(inp["pos3"], np.float32)
    e1 = [_build_E(pos1s, pi, True) for pi in range(2)]
    e2 = [_build_E(pos2s, pi, False) for pi in range(2)]

    qpos = _qpos(inp["sentence_lengths"])
    g3 = []
    for b in range(4):
        idx = np.clip(np.arange(LK) - int(qpos[b]), -M, M) + M
        g = pos3[idx].T.astype(BF16)          # [64, LK]
        g3.append(np.concatenate([g, g], axis=0))  # [128, LK] duplicated

    x = np.asarray(inp["self_input"], np.float32)
    ch = np.asarray(inp["char_enc"], np.float32)
    wd = np.asarray(inp["word_enc"], np.float32)

    in_maps = []
    for core in range(8):
        b, pi = core // 2, core % 2
        rows = _own_rows(pi)
        xT = np.ascontiguousarray(x[b].T)            # [D, 1024]
        m = dict(weights)
        m["xow"] = np.ascontiguousarray(xT[:, rows])
        m["xob"] = m["xow"].astype(BF16)
        m["sfb"] = xT.astype(BF16)
        m["chb"] = np.ascontiguousarray(ch[b].T).astype(BF16)
        m["wdb"] = np.ascontiguousarray(wd[b].T).astype(BF16)
        m["e1t"] = e1[pi][0]
        m["e1m"] = e1[pi][1]
        m["e2t"] = e2[pi][0]
        m["g3t"] = g3[b]
        in_maps.append(m)
    return in_maps


def _fast_path_ok(inp):
    lam = np.asarray(inp["look_ahead_mask"])
    B, Lq = 4, 1024
    if lam.shape != (1, 1, Lq, Lq):
        return False
    causal = np.triu(np.ones((Lq, Lq), bool), k=1)
    if not np.array_equal(lam[0, 0].astype(bool), causal):
        return False
    if np.asarray(inp["char_mask"]).any() or np.asarray(inp["word_mask"]).any():
        return False
    if np.asarray(inp["sentence_lengths"]).sum() != B:
        return False
    return True


def _numpy_reference(inp):
    """Pure-numpy fallback (slow but exact) for unexpected mask patterns."""
    f = lambda k: np.asarray(inp[k], np.float32)

    def ln(x, g, b):
        m = x.mean(-1, keepdims=True)
        v = ((x - m) ** 2).mean(-1, keepdims=True)
        return (x - m) / np.sqrt(v + EPS) * g + b

    def split_heads(x):
        B, S, _ = x.shape
        return x.reshape(B, S, H, DH).transpose(0, 2, 1, 3)

    def softmax(x):
        x = x - x.max(-1, keepdims=True)
        e = np.exp(x)
        return e / e.sum(-1, keepdims=True)

    def attn(Q, K, V, pl, mask):
        logits = (np.einsum('bhid,bhjd->bhij', Q, K) + pl) / SCALE
        logits = np.where(mask, -np.inf, logits)
        p = softmax(logits)
        out = np.einsum('bhij,bhjd->bhid', p, V)
        B, h, S, dh = out.shape
        return out.transpose(0, 2, 1, 3).reshape(B, S, h * dh)

    def char_pos(emb, lq, lk):
        idx = np.clip(np.arange(lk)[None, :] - np.arange(lq)[:, None], -M, M) + M
        return emb[idx]

    x0 = f("self_input")
    B, Lq, _ = x0.shape
    qkv = (x0 @ f("qkv_w") + f("qkv_b")).reshape(B, Lq, H, 3 * DH).transpose(0, 2, 1, 3)
    Q, K, V = np.split(qkv, 3, axis=-1)
    pl = np.einsum('bhid,ijd->bhij', Q, char_pos(f("pos1"), Lq, Lq))
    a = attn(Q, K, V, pl, np.asarray(inp["look_ahead_mask"])) @ f("o1_w") + f("o1_b")
    x = ln(a + x0, f("ln1_g"), f("ln1_b"))

    ce = f("char_enc")
    Q = split_heads(x @ f("q2_w") + f("q2_b"))
    K = split_heads(ce @ f("k2_w") + f("k2_b"))
    V = split_heads(ce @ f("v2_w") + f("v2_b"))
    pl = np.einsum('bhid,ijd->bhij', Q, char_pos(f("pos2"), Lq, ce.shape[1]))
    a = attn(Q, K, V, pl, np.asarray(inp["char_mask"])) @ f("o2_w") + f("o2_b")
    x = ln(a + x, f("ln2_g"), f("ln2_b"))

    we = f("word_enc")
    Q = split_heads(x @ f("q3_w") + f("q3_b"))
    K = split_heads(we @ f("k3_w") + f("k3_b"))
    V = split_heads(we @ f("v3_w") + f("v3_b"))
    qpos = _qpos(inp["sentence_lengths"])
    idx = np.clip(np.arange(we.shape[1])[None, :] - qpos[:, None], -M, M) + M
    pl = np.einsum('bhid,bjd->bhij', Q, f("pos3")[idx])
    a = attn(Q, K, V, pl, np.asarray(inp["word_mask"])) @ f("o3_w") + f("o3_b")
    x = ln(a + x, f("ln3_g"), f("ln3_b"))

    ffn = np.maximum(x @ f("f1_w") + f("f1_b"), 0.0) @ f("f2_w") + f("f2_b")
    return ln(ffn + x, f("ln3_g"), f("ln3_b"))


def kernel(**inputs) -> np.ndarray:
    if not _fast_path_ok(inputs):
        return _numpy_reference(inputs)
    nc = _get_nc()
    in_maps = _host_prep(inputs)
    res = run_bass_kernel_spmd(nc, in_maps, list(range(8)))
    y = np.empty((4, 1024, 1024), np.float32)
    for core in range(8):
        b, pi = core // 2, core % 2
        yT = res.results[core]["yT"]
        y[b, _own_rows(pi), :] = yT.T
    return y

